# revision 22
# baseline (speedup 1.0000x reference)
"""Grouped Conv2D (32 groups of 8->8 ch, 3x3, SAME) on 8 trn2 NeuronCores.

Strategy (v4 - dense-contraction implicit GEMM, group-sharded):
  - Shard over channel GROUPS: each core owns 4 of the 32 groups for all 32
    images (expert-style parallelism). Same FLOPs/bytes for activations as
    batch sharding, but per-core weight traffic drops 8x (0.34 MB).
  - Per group g, pack 14 consecutive output ROWS into PE output partitions:
      po = (oc, pr)  : 8 out-ch x 14 rows = 112 outputs
      pc = (ic, rho) : 8 in-ch x 16 input rows = 128 -> contraction fully dense
    The 3 column-taps (tw) are 3 accumulating matmuls with col-shifted rhs
    views; the 3 row-taps live inside the (rho - pr) Toeplitz structure of the
    stationary. 192 matmuls of [128x128 @ 128x448] per core vs 504 for the
    16-group block-diagonal scheme (2.6x fewer PE columns).
  - Host pre-packs input into [pc, pair, g, (img,rgrp), col] bf16 so every DMA
    is a dense [128 x contiguous] rectangle; output returned bf16 in matmul
    layout and unscrambled + upcast on host.
  - All DMAs ride one sync HWDGE ring: inputs first, outputs FIFO behind them,
    so input feed has strict HBM priority and completions stay on the cheap
    HWDGE path.
"""

import sys

import numpy as np

if "/opt/trn_rl_repo" not in sys.path:
    sys.path.insert(0, "/opt/trn_rl_repo")

import ml_dtypes

B, C, H, W = 32, 256, 56, 56
KK = 3
GROUPS = 32
CPG = 8  # in- and out-channels per group
N_CORES = 8
GPC = GROUPS // N_CORES  # groups per core
NPAIR = B // 2  # image pairs per core (all 32 images, 16 pairs)
PR = 14  # output rows per partition-block
RG = 4  # row groups (4 x 14 = 56 rows)
RHO = 16  # input rows per row group (14 + 2 halo)
PO = CPG * PR  # 112 output partitions
FREE = 2 * RG * W  # 448 matmul columns: (img, rgrp, col)
WP = W + 2  # padded cols


def _pack_weights(w: np.ndarray) -> np.ndarray:
    """[256, 8, 3, 3] fp32 -> [128 pc, 32 g, 3 tw, 112 po] bf16 block-Toeplitz.

    wpk[(ic,rho), g, tw, (oc,pr)] = w[8g+oc, ic, rho-pr, tw] for rho-pr in 0..2
    """
    wr = w.reshape(GROUPS, CPG, CPG, KK, KK)  # g, oc, ic, th, tw
    wl = np.zeros((CPG, RHO, GROUPS, KK, CPG, PR), np.float32)
    for th in range(KK):
        src = wr[:, :, :, th, :].transpose(2, 0, 3, 1)  # ic, g, tw, oc
        for pr in range(PR):
            wl[:, pr + th, :, :, :, pr] = src
    return wl.reshape(128, GROUPS, KK, PO).astype(ml_dtypes.bfloat16)


def _pack_inputs(x: np.ndarray) -> np.ndarray:
    """[32, 256, 56, 56] fp32 -> [8 core, 128 pc, 16 pair, 4 g, 8 m, 58 c] bf16.

    xin[core, (ic,rho), pair, g, (img,rgrp), c] = xpad[b, ch, 14*rgrp+rho, c]
    with b = 2*pair + img, ch = 8*(4*core + g) + ic, xpad zero-padded by 1.
    """
    xpad = np.zeros((B, C, H + 2, WP), dtype=ml_dtypes.bfloat16)
    xpad[:, :, 1 : H + 1, 1 : W + 1] = x.astype(ml_dtypes.bfloat16)
    rows = PR * np.arange(RG)[:, None] + np.arange(RHO)[None, :]  # [rgrp, rho]
    xg = xpad.reshape(NPAIR, 2, N_CORES, GPC, CPG, H + 2, WP)
    xr = xg[:, :, :, :, :, rows, :]  # pair, img, core, g, ic, rgrp, rho, c
    return np.ascontiguousarray(
        xr.transpose(2, 4, 6, 0, 3, 1, 5, 7).reshape(
            N_CORES, 128, NPAIR, GPC, 2 * RG, WP
        )
    )


def _unpack_output(outs) -> np.ndarray:
    """per-core [112, 16, 4, 448] bf16 -> [32, 256, 56, 56] fp32."""
    o = np.stack([np.asarray(t) for t in outs])
    o = o.reshape(N_CORES, CPG, PR, NPAIR, GPC, 2, RG, W)
    o = o.transpose(3, 5, 0, 4, 1, 6, 2, 7)  # pair,img,core,g,oc,rgrp,pr,c
    return np.ascontiguousarray(o.reshape(B, C, H, W)).astype(np.float32)


def _build_bass():
    import concourse.tile as tile
    from concourse import bacc, mybir

    nc = bacc.Bacc()
    xin = nc.dram_tensor(
        "xin", [128, NPAIR, GPC, 2 * RG, WP], mybir.dt.bfloat16,
        kind="ExternalInput",
    )
    wpk = nc.dram_tensor(
        "wpk", [128, GPC, KK, PO], mybir.dt.bfloat16, kind="ExternalInput"
    )
    out = nc.dram_tensor(
        "out", [PO, NPAIR, GPC, FREE], mybir.dt.bfloat16, kind="ExternalOutput"
    )

    # input chunking along image pairs: everything on the sync HWDGE ring,
    # outputs FIFO behind inputs; chunks sized so the stream stays ahead of
    # the PE while minimizing per-DMA issue overhead.
    PCHUNKS = [(1, 2), (2, 4), (4, 7), (7, 11), (11, 16)]

    with tile.TileContext(nc) as tc:
        with (
            tc.tile_pool(name="singles", bufs=1) as singles,
            tc.tile_pool(name="psum_pool", bufs=8, space="PSUM") as psum_pool,
        ):
            X = singles.tile([128, NPAIR, GPC, 2 * RG, WP], mybir.dt.bfloat16)
            Wt = singles.tile([128, GPC, KK, PO], mybir.dt.bfloat16)
            O = singles.tile([PO, NPAIR, GPC, FREE], mybir.dt.bfloat16)

            # PE warm-up on a memset tile so the HAM clock gate (1.2 ->
            # 2.4 GHz) starts releasing before the first weights even land.
            wz = singles.tile([128, 128], mybir.dt.bfloat16)
            nc.vector.memset(wz[:], 0)
            wu = psum_pool.tile([128, FREE], mybir.dt.float32, name="ps")
            for _ in range(12):
                nc.tensor.matmul(
                    wu[:, :128], lhsT=wz[:], rhs=wz[:], start=True, stop=True
                )

            # tiny first loads so the first real matmul starts ~2us earlier
            nc.sync.dma_start(out=X[:, 0, 0:1], in_=xin[:, 0, 0:1])
            nc.sync.dma_start(out=Wt[:, 0:1], in_=wpk[:, 0:1])
            nc.sync.dma_start(out=X[:, 0, 1:], in_=xin[:, 0, 1:])
            nc.sync.dma_start(out=Wt[:, 1:], in_=wpk[:, 1:])
            for a, b in PCHUNKS:
                nc.sync.dma_start(out=X[:, a:b], in_=xin[:, a:b])

            # output chunks: big early (amortize per-DMA issue+completion,
            # ~1.7us each on the ring), small late (shortest exposed tail).
            # all copies of a chunk ride ONE engine so each output DMA has a
            # single sync wait.
            OCHUNKS = [(0, 8), (8, 12), (12, 14), (14, 15), (15, 16)]
            copy_engines = [nc.scalar, nc.vector]
            for ci, (pa, pb) in enumerate(OCHUNKS):
                last = ci == len(OCHUNKS) - 1
                for pair in range(pa, pb):
                    for g in range(GPC):
                        eng = copy_engines[(g // 2) if last else (ci % 2)]
                        ps = psum_pool.tile([128, FREE], mybir.dt.float32, name="ps")
                        for tw in range(KK):
                            nc.tensor.matmul(
                                ps[:PO, :],
                                lhsT=Wt[:, g, tw, :],
                                rhs=X[:, pair, g, :, tw : tw + W],
                                start=(tw == 0),
                                stop=(tw == KK - 1),
                            )
                        if eng is nc.scalar:
                            eng.copy(out=O[:, pair, g], in_=ps[:PO, :])
                        else:
                            eng.tensor_copy(out=O[:, pair, g], in_=ps[:PO, :])
                if last:
                    # final pair: copies split across both engines, two half
                    # DMAs so the first half drains during the last copies
                    nc.sync.dma_start(out=out[:, pa, 0:2], in_=O[:, pa, 0:2])
                    nc.sync.dma_start(out=out[:, pa, 2:4], in_=O[:, pa, 2:4])
                else:
                    nc.sync.dma_start(out=out[:, pa:pb], in_=O[:, pa:pb])
    nc.finalize()
    return nc


_CACHE = {}


def kernel(x, w, trace=False):
    from concourse.bass_utils import run_bass_kernel_spmd

    x = np.ascontiguousarray(np.asarray(x), dtype=np.float32)
    w = np.ascontiguousarray(np.asarray(w), dtype=np.float32)

    if "nc" not in _CACHE:
        _CACHE["nc"] = _build_bass()
    nc = _CACHE["nc"]

    xin = _pack_inputs(x)
    wp = _pack_weights(w)
    in_maps = [
        {"xin": xin[i], "wpk": np.ascontiguousarray(wp[:, GPC * i : GPC * (i + 1)])}
        for i in range(N_CORES)
    ]
    res = run_bass_kernel_spmd(
        nc, in_maps, core_ids=list(range(N_CORES)), trace=trace
    )
    outp = _unpack_output([res.results[i]["out"] for i in range(N_CORES)])
    if trace:
        kernel.last_result = res
    return outp
